# revision 42
# baseline (speedup 1.0000x reference)
"""Multi-head causal attention (B=4, S=2048, E=1024, H=16, Dh=64) on 8
Trainium2 NeuronCores.

Sharding: data-parallel over the 4 batch elements x tensor-parallel over
heads (2 groups of 8). Core 2b+g handles batch b, heads 8g..8g+7. Each core
computes Q^T/K^T (head dim on partitions), V (natural layout, with a fused
ones-column so the attention-weight row sums fall out of the same matmul),
block-causal scores in transposed [kv, q] layout (so no transposes are ever
needed: softmax normalization is a reciprocal + partition-broadcast), the
local-head context, and the output projection against its slice of Wo. The
two partial projections per batch are summed on the host (the TP
"all-reduce" of the sharding hint, done at gather time), which also absorbs
the out-transpose: the kernel emits out^T [E, S].

All matmuls run in bfloat16 (fp32 accumulate): same PE rate as float32r but
half the SBUF/DMA/LDWEIGHTS traffic and lower PE power draw. Power draw
matters directly here: the chip's activity throttle (HAM util cap, k=4/8)
engages after ~90us of sustained full-width matmul and halves PE duty. The
f32r version lost ~170us to it; bf16 loses ~60us. Stage B's instruction mix
(64-contraction scores, per-matmul LDWEIGHTS gaps) keeps it just under the
throttle's release threshold — deliberately denser variants (fused exps,
eager LDW dedup inside stage B) measured SLOWER because the throttle then
never releases. Hence LDWEIGHTS dedup only fires where the natural
instruction order creates back-to-back duplicates (stages A and C).
"""

import json
import os
import sys

for _p in ("/opt/trn_rl_repo",):
    if _p not in sys.path:
        sys.path.insert(0, _p)

import numpy as np

# ---------------------------------------------------------------- constants
B = 4
S = 2048
E = 1024
H = 16
DH = 64
HL = 8  # heads per core
DL = HL * DH  # 512, local head dim
P = 128
NCORES = 8
SCALE = 1.0 / 8.0  # 1/sqrt(DH)
NEG = -1.0e30

KT_E = E // P  # 8  k-tiles over embed dim
MT = DL // P  # 4  m-tiles over local head dim (2 heads per m-tile)
SC = S // 512  # 4  512-wide chunks over sequence
SB = S // P  # 16 128-blocks over sequence
KT_D = DL // P  # 4  k-tiles over local head dim (proj contraction)
MT_E = E // P  # 8  m-tiles over embed dim (proj output)
VW = DH + 1  # 65: V columns per head + ones column


# ------------------------------------------------- BIR post-processing
# 1. Ldweights dedup: bass splits every bf16 matmul into Ldweights+Matmult
#    (walrus ldw-opt rejects these pairs, so the elision happens here).
#    Only back-to-back identical loads are elided, which by construction
#    only exist in stages A and C — stage B's per-matmul loads are kept as
#    deliberate pacing (see module docstring).
# 2. Multi-wait splitting: this walrus accepts one sync-wait per
#    instruction; extras become single-wait EventSemaphores.
_syncfix_done = [False]


def _install_syncfix():
    if _syncfix_done[0]:
        return
    _syncfix_done[0] = True
    import concourse.bass_utils as bu

    counter = [0]

    def dedup_ldweights(d):
        changed = False
        for fn in d.get("functions", []):
            for bb in fn.get("blocks", []):
                last_sig = None
                pending = []
                out = []
                for inst in bb.get("instructions", []):
                    if inst.get("engine") != "PE":
                        out.append(inst)
                        continue
                    op = inst["opcode"]
                    if op == "Ldweights":
                        sig = json.dumps(
                            [
                                inst.get("ins"),
                                inst.get("tile_position"),
                                inst.get("tile_size"),
                            ],
                            sort_keys=True,
                        )
                        if sig == last_sig and not (
                            (inst.get("sync_info") or {}).get("on_update")
                        ):
                            changed = True
                            w = (inst.get("sync_info") or {}).get("on_wait") or []
                            pending.extend(w)
                            continue
                        last_sig = sig
                    elif op in ("Matmult", "EventSemaphore"):
                        pass  # PE weight state preserved
                    else:
                        last_sig = None
                    if pending:
                        si = inst.setdefault(
                            "sync_info", {"on_update": [], "on_wait": []}
                        )
                        si["on_wait"] = list(si.get("on_wait") or []) + pending
                        pending = []
                    out.append(inst)
                bb["instructions"] = out
        return changed

    def inject_pacing(d):
        """Insert idempotent PE no-ops (RegisterMove PE_zero<-0) after every
        Nth full-width projection matmul. Stages A and C run 128-contraction
        matmuls back to back at ~100% PE-array duty, which charges the
        chip's power throttle and costs ~50% duty caps that bleed far into
        stage B. A ~10% duty gap keeps the integrator under its trip point.
        Stage-B matmuls (64-contraction, naturally padded by LDWEIGHTS) are
        left untouched."""
        # Pacing is OFF by default: measured no effect on the throttle trip
        # point from either mechanism (RegisterMoves vanish in the
        # HW-decoded stream; EventSemaphore re-waits cost ~55ns each but the
        # ~11% duty dilution did not shift the ~115us trip at all — the
        # controller appears thermal/chip-global with long time constants).
        pace_a = int(os.environ.get("KPACE_A", "0"))
        pace_c = int(os.environ.get("KPACE_C", "0"))
        if pace_a <= 0 and pace_c <= 0:
            return False
        a_pref = ("wq_sb", "wk_sb", "wv_sb")
        n = [0]
        cnt = {"A": 0, "C": 0}

        sem_mode = os.environ.get("KPACE_MODE", "sem") == "sem"

        def noop(wait):
            n[0] += 1
            if sem_mode and wait is not None:
                # EventSemaphore re-waiting an already-satisfied DMA counter
                # threshold: ~55ns of real PE-sequencer time (RegisterMoves
                # vanish in the HW-decoded stream). DMA counters only
                # increment, so a repeated sem-ge wait can never block.
                return {
                    "debug": 0,
                    "engine": "PE",
                    "ins": [],
                    "name": f"PACE-{n[0]}",
                    "opcode": "EventSemaphore",
                    "outs": [],
                    "sync_info": {"on_update": [], "on_wait": [wait]},
                }
            return {
                "debug": 0,
                "engine": "PE",
                "ins": [{"dtype": "int32", "kind": "imm_value", "value": 0}],
                "name": f"PACE-{n[0]}",
                "opcode": "RegisterMove",
                "outs": [
                    {
                        "dtype": "int32",
                        "kind": "register_access",
                        "regref": "PE_zero",
                    }
                ],
            }

        changed = False
        for fn in d.get("functions", []):
            for bb in fn.get("blocks", []):
                out = []
                last_wait = [None]
                for inst in bb.get("instructions", []):
                    out.append(inst)
                    if inst.get("engine") != "PE":
                        continue
                    for w in (inst.get("sync_info") or {}).get("on_wait") or []:
                        if (
                            w.get("wait_mode") == "sem-ge-imm"
                            and "DMA" in (w.get("ant_name") or "")
                        ):
                            last_wait[0] = w
                    if inst["opcode"] != "Matmult":
                        continue
                    refs = [
                        i.get("memref", "")
                        for i in inst.get("ins", [])
                        if isinstance(i, dict)
                    ]
                    grp = None
                    if any(r.startswith(a_pref) for r in refs):
                        grp = ("A", pace_a)
                    elif any(r.startswith("wo_sb") for r in refs):
                        grp = ("C", pace_c)
                    if grp is None or grp[1] <= 0:
                        continue
                    cnt[grp[0]] += 1
                    if cnt[grp[0]] % grp[1] == 0:
                        out.append(noop(last_wait[0]))
                        changed = True
                bb["instructions"] = out
        return changed

    def split_multiwait(bir_json):
        d = json.loads(bir_json)
        changed = dedup_ldweights(d) if os.environ.get("KDEDUP", "1") == "1" else False
        changed |= inject_pacing(d)
        for fn in d.get("functions", []):
            for bb in fn.get("blocks", []):
                new_insts = []
                for inst in bb.get("instructions", []):
                    si = inst.get("sync_info")
                    waits = (si or {}).get("on_wait") or []
                    if len(waits) > 1:
                        changed = True
                        for w in waits[:-1]:
                            counter[0] += 1
                            new_insts.append(
                                {
                                    "debug": inst.get("debug"),
                                    "engine": inst["engine"],
                                    "ins": [],
                                    "name": f"WSPLIT-{counter[0]}",
                                    "opcode": "EventSemaphore",
                                    "outs": [],
                                    "sync_info": {"on_update": [], "on_wait": [w]},
                                }
                            )
                        si["on_wait"] = [waits[-1]]
                    new_insts.append(inst)
                bb["instructions"] = new_insts
        if not changed:
            return bir_json if isinstance(bir_json, bytes) else bir_json.encode()
        return json.dumps(d).encode()

    orig = bu.compile_bir_kernel

    def patched(bir_json, tmpdir, neff_name="file.neff"):
        return orig(split_multiwait(bir_json), tmpdir, neff_name)

    bu.compile_bir_kernel = patched
    try:
        import concourse.bass2jax as b2j

        if hasattr(b2j, "compile_bir_kernel"):
            b2j.compile_bir_kernel = patched
    except ImportError:
        pass


# ------------------------------------------------------------ kernel build
def build_nc():
    import concourse.bass as bass
    import concourse.tile as tile
    from concourse import mybir

    f32 = mybir.dt.float32
    f32r = mybir.dt.bfloat16  # value-path dtype (name kept from f32r version)
    EXP = mybir.ActivationFunctionType.Exp
    LN = mybir.ActivationFunctionType.Ln
    IDENT = mybir.ActivationFunctionType.Identity

    nc = bass.Bass()

    xt_ext = nc.dram_tensor("xt", [E, S], f32r, kind="ExternalInput")
    wq_ext = nc.dram_tensor("wq", [E, DL], f32r, kind="ExternalInput")
    wk_ext = nc.dram_tensor("wk", [E, DL], f32r, kind="ExternalInput")
    wv_ext = nc.dram_tensor("wv", [E, DL], f32r, kind="ExternalInput")
    wo_ext = nc.dram_tensor("wo", [DL, E], f32r, kind="ExternalInput")
    bo_ext = nc.dram_tensor("bo2", [E], f32, kind="ExternalInput")
    mask_ext = nc.dram_tensor("mask", [P, P], f32, kind="ExternalInput")
    vones_ext = nc.dram_tensor("vones", [P, SB * HL], f32r, kind="ExternalInput")
    o64_ext = nc.dram_tensor("o64", [P, DH], f32r, kind="ExternalInput")
    out_ext = nc.dram_tensor("outp", [E, S], f32r, kind="ExternalOutput")

    xt_r = xt_ext.rearrange("(kt p) s -> p kt s", p=P)
    wq_r = wq_ext.rearrange("(kt p) d -> p kt d", p=P)
    wk_r = wk_ext.rearrange("(kt p) d -> p kt d", p=P)
    wv_r = wv_ext.rearrange("(kt p) d -> p kt d", p=P)
    wo_r = wo_ext.rearrange("(kt p) e -> p kt e", p=P)
    bo_r = bo_ext.rearrange("(m p) -> p m", p=P)

    with tile.TileContext(nc) as tc:
        with tc.tile_pool(name="persist", bufs=1) as pers:
            # ---- persistent SBUF tensors
            qt = [pers.tile([P, S], f32r, tag=f"qt{m}", name=f"qt{m}") for m in range(MT)]
            kt = [pers.tile([P, S], f32r, tag=f"kt{m}", name=f"kt{m}") for m in range(MT)]
            v_sb = pers.tile([P, SB, HL * VW], f32r, tag="v")
            bo_sb = pers.tile([P, MT_E], f32, tag="bo")
            mask_sb = pers.tile([P, P], f32, tag="mask")
            ones64 = pers.tile([P, DH], f32r, tag="ones64")

            # ---- stage A: QT/KT (transposed) and V (natural) projections.
            # Chunk pairs keep SBUF under budget while letting each weight
            # tile serve 2 consecutive matmuls (Ldweights dedup elides the
            # second load).
            with (
                tc.tile_pool(name="wqkv", bufs=1) as wpool,
                tc.tile_pool(name="xt", bufs=3) as xpool,
                tc.tile_pool(name="ps_a", bufs=6, space="PSUM") as ps_a,
            ):
                wq_sb = wpool.tile([P, KT_E, DL], f32r, tag="wq")
                wk_sb = wpool.tile([P, KT_E, DL], f32r, tag="wk")
                wv_sb = wpool.tile([P, KT_E, DL], f32r, tag="wv")
                # DMA issue order matters: each queue drains FIFO, and the
                # gpsimd queue is software-DGE (~5x slower) — putting wq
                # there gated the first matmul at ~15us. wq/wk ride the two
                # hardware queues right behind the xc0 halves; only
                # late-needed tensors (wv, small stuff) go software.
                # Startup is chip-HBM-bound (8 cores pull inputs at once),
                # so only gate-critical bytes go first: the first QK matmul
                # pair needs xc0+xc1 (chunk-pair, c-inner loop) and wq's
                # m=0 slice. Everything else queues behind them.
                xt_tiles = {}
                xc0 = xpool.tile([P, KT_E, 512], f32r, tag="xt", name="xt0")
                xc1 = xpool.tile([P, KT_E, 512], f32r, tag="xt", name="xt1")
                nc.sync.dma_start(xc0[:, 0 : KT_E // 2, :], xt_r[:, 0 : KT_E // 2, 0:512])
                nc.scalar.dma_start(
                    xc0[:, KT_E // 2 :, :], xt_r[:, KT_E // 2 :, 0:512]
                )
                nc.sync.dma_start(
                    xc1[:, 0 : KT_E // 2, :], xt_r[:, 0 : KT_E // 2, 512:1024]
                )
                nc.scalar.dma_start(
                    xc1[:, KT_E // 2 :, :], xt_r[:, KT_E // 2 :, 512:1024]
                )
                xt_tiles[0] = xc0
                xt_tiles[1] = xc1
                nc.sync.dma_start(wq_sb[:, :, 0:P], wq_r[:, :, 0:P])
                nc.scalar.dma_start(wq_sb[:, :, P:DL], wq_r[:, :, P:DL])
                nc.sync.dma_start(wk_sb[:], wk_r)
                nc.gpsimd.dma_start(wv_sb[:], wv_r)

                for cp in range(SC // 2):
                    cs = (2 * cp, 2 * cp + 1)
                    xt_sb = {}
                    for c in cs:
                        if c in xt_tiles:
                            xt_sb[c] = xt_tiles[c]
                            continue
                        xc = xpool.tile(
                            [P, KT_E, 512], f32r, tag="xt", name=f"xt{c}"
                        )
                        nc.sync.dma_start(
                            xc[:], xt_r[:, :, 512 * c : 512 * (c + 1)]
                        )
                        xt_sb[c] = xc

                    for w_sb, dst in ((wq_sb, qt), (wk_sb, kt)):
                        for m in range(MT):
                            pss = {
                                c: ps_a.tile(
                                    [P, 512], f32, tag="ps_a", name=f"psa{m}_{c}"
                                )
                                for c in cs
                            }
                            for k in range(KT_E):
                                for c in cs:
                                    nc.tensor.matmul(
                                        pss[c][:],
                                        w_sb[:, k, P * m : P * (m + 1)],
                                        xt_sb[c][:, k, :],
                                        start=(k == 0),
                                        stop=(k == KT_E - 1),
                                    )
                            for c in cs:
                                nc.vector.tensor_copy(
                                    dst[m][:, 512 * c : 512 * (c + 1)], pss[c][:]
                                )

                    # V natural: psum[sb] = sum_k XT[k, sblock].T @ WV[k, :]
                    for c in cs:
                        for s in range(4):
                            sb = 4 * c + s
                            ps = ps_a.tile(
                                [P, 512], f32, tag="ps_a", name=f"psv{sb}"
                            )
                            for k in range(KT_E):
                                nc.tensor.matmul(
                                    ps[:],
                                    xt_sb[c][:, k, P * s : P * (s + 1)],
                                    wv_sb[:, k, :],
                                    start=(k == 0),
                                    stop=(k == KT_E - 1),
                                )
                            for h in range(HL):
                                nc.vector.tensor_copy(
                                    v_sb[:, sb, VW * h : VW * h + DH],
                                    ps[:, DH * h : DH * (h + 1)],
                                )

            # small/strided loads issued after the stage-A streams so they
            # don't delay the HWDGE queues feeding the first matmuls
            nc.gpsimd.dma_start(bo_sb[:], bo_r)
            nc.gpsimd.dma_start(mask_sb[:], mask_ext[:, :])
            nc.gpsimd.dma_start(ones64[:], o64_ext[:])
            # ones column for the fused row-sum in the AV matmul (DMA'd in:
            # memset can't write 1.0 into a bf16 tensor on this compiler)
            ones_col = v_sb[:].rearrange("p sb (h c) -> p sb h c", c=VW)[
                :, :, :, DH : DH + 1
            ]
            nc.sync.dma_start(
                ones_col, vones_ext.rearrange("p (sb h one) -> p sb h one", h=HL, one=1)
            )

            # ---- stage B: block-causal attention in transposed layout
            with tc.tile_pool(name="late", bufs=1) as late:
                # allocated after the stage-A pools are gone (SBUF budget);
                # Wo prefetches while attention runs
                ct = [
                    late.tile([P, KT_D, 512], f32r, tag=f"ct{i}", name=f"ct{i}")
                    for i in range(SC)
                ]
                wo_sb = late.tile([P, KT_D, E], f32r, tag="wo")
                nc.sync.dma_start(wo_sb[:], wo_r)
                _stage_b(
                    nc, tc, qt, kt, v_sb, ct, mask_sb, ones64, EXP, LN, f32, f32r
                )

                # ---- stage C: out^T = WoT-slice.T @ ctx^T (+ bias, group 0)
                with (
                    tc.tile_pool(name="ostage", bufs=4) as opool,
                    tc.tile_pool(name="ps_p", bufs=8, space="PSUM") as ps_p,
                ):
                    for m in range(MT_E):
                        pss = [
                            ps_p.tile([P, 512], f32, tag="ps_p", name=f"psp{m}_{s}")
                            for s in range(SC)
                        ]
                        for k in range(KT_D):
                            for s in range(SC):
                                nc.tensor.matmul(
                                    pss[s][:],
                                    wo_sb[:, k, P * m : P * (m + 1)],
                                    ct[s][:, k, :],
                                    start=(k == 0),
                                    stop=(k == KT_D - 1),
                                )
                        for s in range(SC):
                            ot = opool.tile(
                                [P, 512], f32r, tag="ostage", name=f"ot{m}_{s}"
                            )
                            nc.scalar.activation(
                                ot[:], pss[s][:], IDENT, bias=bo_sb[:, m : m + 1]
                            )
                            nc.sync.dma_start(
                                out_ext[
                                    P * m : P * (m + 1), 512 * s : 512 * (s + 1)
                                ],
                                ot[:],
                            )

    return nc


def _stage_b(nc, tc, qt, kt, v_sb, ct, mask_sb, ones64, EXP, LN, f32, f32r):
    with (
        tc.tile_pool(name="probs", bufs=10) as ppool,
        tc.tile_pool(name="fin", bufs=2) as fpool,
        tc.tile_pool(name="ps_s", bufs=3, space="PSUM") as ps_s,
        tc.tile_pool(name="ps_ctx", bufs=3, space="PSUM") as ps_ctx,
        tc.tile_pool(name="ps_bc", bufs=2, space="PSUM") as ps_bc,
    ):
        for c in range(SC):
            q_lo, q_hi = 512 * c, 512 * (c + 1)
            # row sums at 32-aligned partitions {0,32} x 4 free slots so the
            # broadcast matmul can read the reciprocals in place (rhs/lhsT
            # base partition must be 0/32/64)
            sums_sb = fpool.tile([P, 2048], f32, tag="sums", name=f"sums{c}")
            for t in range(MT):  # head pair (2t, 2t+1)
                heads = (2 * t, 2 * t + 1)
                nblk = 4 * c + 4
                ctx_ps = {}
                probs = {}
                for h in heads:
                    ctx_ps[h] = ps_ctx.tile([VW, 512], f32, tag="ps_ctx", name=f"ctx_{c}_{h}")

                def av_mm(h, j, first, last):
                    r = j - 4 * c
                    lo = P * r if r > 0 else 0
                    nc.tensor.matmul(
                        ctx_ps[h][:, lo:512],
                        v_sb[:, j, VW * h : VW * (h + 1)],
                        probs[(h, j)][:, lo:512],
                        start=first,
                        stop=last,
                    )

                for j in range(nblk):
                    r = j - 4 * c  # >=0 only on diagonal band
                    for h in heads:
                        hp = DH * (h % 2)
                        ps = ps_s.tile([P, 512], f32, tag="ps_s")
                        # scoresT[kv_block j, q chunk c] = K_j @ Q^T; on the
                        # diagonal band only cols >= 128r are causally live.
                        # bf16 has no small-N rate penalty, so trim all of it.
                        slo = P * r if r > 0 else 0
                        nc.tensor.matmul(
                            ps[:, slo:512],
                            kt[t][hp : hp + DH, P * j : P * (j + 1)],
                            qt[t][hp : hp + DH, q_lo + slo : q_hi],
                            start=True,
                            stop=True,
                        )
                        if r >= 0:
                            # triangle mask on the diagonal 128x128
                            nc.vector.tensor_add(
                                ps[:, P * r : P * (r + 1)],
                                ps[:, P * r : P * (r + 1)],
                                mask_sb[:],
                            )
                        pr = ppool.tile(
                            [P, 512], f32r, tag="probs", name=f"pr_{c}_{h}_{j}"
                        )
                        probs[(h, j)] = pr
                        lo = P * r if r > 0 else 0
                        nc.scalar.activation(
                            pr[:, lo:512], ps[:, lo:512], EXP, scale=SCALE
                        )
                    if j >= 1:
                        for h in heads:
                            av_mm(h, j - 1, first=(j == 1), last=False)
                for h in heads:
                    av_mm(h, nblk - 1, first=(nblk == 1), last=True)

                # stash unnormalized ctx + row sums
                for h in heads:
                    hp = DH * (h % 2)
                    nc.vector.tensor_copy(
                        ct[c][hp : hp + DH, t, :], ctx_ps[h][0:DH, :]
                    )
                    nc.vector.tensor_copy(
                        sums_sb[
                            32 * (h % 2) : 32 * (h % 2) + 1,
                            512 * (h // 2) : 512 * (h // 2) + 512,
                        ],
                        ctx_ps[h][DH : DH + 1, :],
                    )

            # ---- batched softmax normalization for this chunk: 1/sums as
            # exp(-ln(sums)) on ACT (the DVE-reciprocal variant measured
            # slower overall), written as bf16 directly where the broadcast
            # matmul reads it. bc matmuls are ordered by sums-row parity so
            # the all-ones stationary reloads dedup to one per row.
            lns = fpool.tile([P, 2048], f32, tag="lns", name=f"lns{c}", bufs=1)
            nc.scalar.activation(lns[:], sums_sb[:], LN)
            recs = fpool.tile([P, 2048], f32r, tag="recs", name=f"recs{c}")
            nc.scalar.activation(recs[:], lns[:], EXP, scale=-1.0)
            for h in (0, 2, 4, 6, 1, 3, 5, 7):
                t = h // 2
                hp = DH * (h % 2)
                rp = 32 * (h % 2)
                bc = ps_bc.tile([DH, 512], f32, tag="ps_bc", name=f"bc{c}_{h}")
                nc.tensor.matmul(
                    bc[:],
                    ones64[rp : rp + 1, :],
                    recs[rp : rp + 1, 512 * (h // 2) : 512 * (h // 2) + 512],
                    start=True,
                    stop=True,
                )
                nc.vector.tensor_mul(
                    ct[c][hp : hp + DH, t, :],
                    ct[c][hp : hp + DH, t, :],
                    bc[:],
                )


# ------------------------------------------------------------ PJRT runner
class _Runner:
    """Compile once, run many: mirrors bass2jax.run_bass_via_pjrt with a
    cached jitted executable."""

    def __init__(self, nc):
        import jax
        import jax.numpy  # noqa: F401
        from jax.sharding import Mesh, PartitionSpec
        from jax.experimental.shard_map import shard_map
        import concourse.bass2jax as b2j
        from concourse import mybir

        b2j.install_neuronx_cc_hook()
        self.jax = jax
        partition_name = (
            nc.partition_id_tensor.name if nc.partition_id_tensor else None
        )
        in_names = []
        out_names = []
        out_avals = []
        self.zero_shapes = []
        for alloc in nc.m.functions[0].allocations:
            if not isinstance(alloc, mybir.MemoryLocationSet):
                continue
            name = alloc.memorylocations[0].name
            if alloc.kind == "ExternalInput":
                if name == partition_name:
                    continue
                in_names.append(name)
            elif alloc.kind == "ExternalOutput":
                shape = tuple(alloc.tensor_shape)
                dtype = mybir.dt.np(alloc.dtype)
                out_names.append(name)
                out_avals.append(jax.core.ShapedArray(shape, dtype))
                self.zero_shapes.append((shape, dtype))
        self.in_names = in_names
        self.out_names = out_names
        self.out_avals = out_avals
        n_params = len(in_names)
        n_outs = len(out_avals)
        all_in = list(in_names) + list(out_names)
        if partition_name is not None:
            all_in.append(partition_name)

        def _body(*args):
            operands = list(args)
            if partition_name is not None:
                operands.append(b2j.partition_id_tensor())
            outs = b2j._bass_exec_p.bind(
                *operands,
                out_avals=tuple(out_avals),
                in_names=tuple(all_in),
                out_names=tuple(out_names),
                lowering_input_output_aliases=(),
                sim_require_finite=True,
                sim_require_nnan=True,
                nc=nc,
            )
            return tuple(outs)

        devices = jax.devices()[:NCORES]
        assert len(devices) == NCORES, f"need {NCORES} cores, got {len(devices)}"
        self.mesh = Mesh(np.asarray(devices), ("core",))
        in_specs = (PartitionSpec("core"),) * (n_params + n_outs)
        out_specs = (PartitionSpec("core"),) * n_outs
        self.fn = jax.jit(
            shard_map(
                _body,
                mesh=self.mesh,
                in_specs=in_specs,
                out_specs=out_specs,
                check_rep=False,
            ),
            donate_argnums=tuple(range(n_params, n_params + n_outs)),
            keep_unused=True,
        )

    def run(self, in_maps):
        concat_in = [
            np.concatenate([np.asarray(m[name]) for m in in_maps], axis=0)
            for name in self.in_names
        ]
        zeros = [
            np.zeros((NCORES * s[0], *s[1:]), dt) for s, dt in self.zero_shapes
        ]
        outs = self.fn(*concat_in, *zeros)
        return [
            {
                name: np.asarray(outs[i]).reshape(
                    NCORES, *self.out_avals[i].shape
                )[c]
                for i, name in enumerate(self.out_names)
            }
            for c in range(NCORES)
        ]


_cache = {}


def _get_runner():
    if "runner" not in _cache:
        _install_syncfix()
        _cache["runner"] = _Runner(build_nc())
    return _cache["runner"]


def make_in_maps(X, Wq, Wk, Wv, Wo, bo):
    import ml_dtypes

    bf16 = ml_dtypes.bfloat16
    X = np.asarray(X, dtype=np.float32)
    Wq = np.asarray(Wq, dtype=np.float32)
    Wk = np.asarray(Wk, dtype=np.float32)
    Wv = np.asarray(Wv, dtype=np.float32)
    Wo = np.asarray(Wo, dtype=np.float32)
    bo = np.asarray(bo, dtype=np.float32)

    kv = np.arange(P)[:, None]
    qq = np.arange(P)[None, :]
    mask = np.where(kv > qq, np.float32(NEG), np.float32(0.0))

    in_maps = []
    for core in range(NCORES):
        b, g = divmod(core, 2)
        h0 = HL * g
        in_maps.append(
            {
                "xt": np.ascontiguousarray(X[b].T.astype(bf16)),
                "wq": np.ascontiguousarray(
                    Wq[h0 : h0 + HL].transpose(1, 0, 2).reshape(E, DL).astype(bf16)
                ),
                "wk": np.ascontiguousarray(
                    Wk[h0 : h0 + HL].transpose(1, 0, 2).reshape(E, DL).astype(bf16)
                ),
                "wv": np.ascontiguousarray(
                    Wv[h0 : h0 + HL].transpose(1, 0, 2).reshape(E, DL).astype(bf16)
                ),
                "wo": np.ascontiguousarray(Wo[:, DL * g : DL * (g + 1)].T.astype(bf16)),
                "bo2": bo if g == 0 else np.zeros_like(bo),
                "mask": mask,
                "vones": np.ones((P, SB * HL), dtype=bf16),
                "o64": np.ones((P, DH), dtype=bf16),
            }
        )
    return in_maps


def assemble(results):
    out = np.empty((B, S, E), dtype=np.float32)
    for b in range(B):
        acc = np.asarray(results[2 * b]["outp"], dtype=np.float32) + np.asarray(
            results[2 * b + 1]["outp"], dtype=np.float32
        )
        out[b] = acc.T
    return out


def kernel(X, Wq, Wk, Wv, Wo, bo):
    runner = _get_runner()
    in_maps = make_in_maps(X, Wq, Wk, Wv, Wo, bo)
    results = runner.run(in_maps)
    return assemble(results)


# revision 45
# speedup vs baseline: 1.0816x; 1.0816x over previous
"""Multi-head causal attention (B=4, S=2048, E=1024, H=16, Dh=64) on 8
Trainium2 NeuronCores.

Sharding: data-parallel over the 4 batch elements x tensor-parallel over
heads (2 groups of 8). Core 2b+g handles batch b, heads 8g..8g+7. Each core
computes Q^T/K^T (head dim on partitions), V (natural layout, with a fused
ones-column so the attention-weight row sums fall out of the same matmul),
block-causal scores in transposed [kv, q] layout (so no transposes are ever
needed: softmax normalization is a reciprocal + partition-broadcast), the
local-head context, and the output projection against its slice of Wo. The
two partial projections per batch are summed on the host (the TP
"all-reduce" of the sharding hint, done at gather time), which also absorbs
the out-transpose: the kernel emits out^T [E, S].

All matmuls run in bfloat16 (fp32 accumulate): same PE rate as float32r but
half the SBUF/DMA/LDWEIGHTS traffic and lower PE power draw. Power draw
matters directly here: the chip's activity throttle (HAM util cap, k=4/8)
engages after ~90us of sustained full-width matmul and halves PE duty. The
f32r version lost ~170us to it; bf16 loses ~60us. Stage B's instruction mix
(64-contraction scores, per-matmul LDWEIGHTS gaps) keeps it just under the
throttle's release threshold — deliberately denser variants (fused exps,
eager LDW dedup inside stage B) measured SLOWER because the throttle then
never releases. Hence LDWEIGHTS dedup only fires where the natural
instruction order creates back-to-back duplicates (stages A and C).
"""

import json
import os
import sys

for _p in ("/opt/trn_rl_repo",):
    if _p not in sys.path:
        sys.path.insert(0, _p)

import numpy as np

# ---------------------------------------------------------------- constants
B = 4
S = 2048
E = 1024
H = 16
DH = 64
HL = 8  # heads per core
DL = HL * DH  # 512, local head dim
P = 128
NCORES = 8
SCALE = 1.0 / 8.0  # 1/sqrt(DH)
NEG = -1.0e30

KT_E = E // P  # 8  k-tiles over embed dim
MT = DL // P  # 4  m-tiles over local head dim (2 heads per m-tile)
SC = S // 512  # 4  512-wide chunks over sequence
SB = S // P  # 16 128-blocks over sequence
KT_D = DL // P  # 4  k-tiles over local head dim (proj contraction)
MT_E = E // P  # 8  m-tiles over embed dim (proj output)
VW = DH + 1  # 65: V columns per head + ones column


# ------------------------------------------------- BIR post-processing
# 1. Ldweights dedup: bass splits every bf16 matmul into Ldweights+Matmult
#    (walrus ldw-opt rejects these pairs, so the elision happens here).
#    Only back-to-back identical loads are elided, which by construction
#    only exist in stages A and C — stage B's per-matmul loads are kept as
#    deliberate pacing (see module docstring).
# 2. Multi-wait splitting: this walrus accepts one sync-wait per
#    instruction; extras become single-wait EventSemaphores.
_syncfix_done = [False]


def _install_syncfix():
    if _syncfix_done[0]:
        return
    _syncfix_done[0] = True
    import concourse.bass_utils as bu

    counter = [0]

    def dedup_ldweights(d):
        changed = False
        for fn in d.get("functions", []):
            for bb in fn.get("blocks", []):
                last_sig = None
                pending = []
                out = []
                for inst in bb.get("instructions", []):
                    if inst.get("engine") != "PE":
                        out.append(inst)
                        continue
                    op = inst["opcode"]
                    if op == "Ldweights":
                        sig = json.dumps(
                            [
                                inst.get("ins"),
                                inst.get("tile_position"),
                                inst.get("tile_size"),
                            ],
                            sort_keys=True,
                        )
                        if sig == last_sig and not (
                            (inst.get("sync_info") or {}).get("on_update")
                        ):
                            changed = True
                            w = (inst.get("sync_info") or {}).get("on_wait") or []
                            pending.extend(w)
                            continue
                        last_sig = sig
                    elif op in ("Matmult", "EventSemaphore"):
                        pass  # PE weight state preserved
                    else:
                        last_sig = None
                    if pending:
                        si = inst.setdefault(
                            "sync_info", {"on_update": [], "on_wait": []}
                        )
                        si["on_wait"] = list(si.get("on_wait") or []) + pending
                        pending = []
                    out.append(inst)
                bb["instructions"] = out
        return changed

    def inject_pacing(d):
        """Insert idempotent PE no-ops (RegisterMove PE_zero<-0) after every
        Nth full-width projection matmul. Stages A and C run 128-contraction
        matmuls back to back at ~100% PE-array duty, which charges the
        chip's power throttle and costs ~50% duty caps that bleed far into
        stage B. A ~10% duty gap keeps the integrator under its trip point.
        Stage-B matmuls (64-contraction, naturally padded by LDWEIGHTS) are
        left untouched."""
        # Pacing is OFF by default: measured no effect on the throttle trip
        # point from either mechanism (RegisterMoves vanish in the
        # HW-decoded stream; EventSemaphore re-waits cost ~55ns each but the
        # ~11% duty dilution did not shift the ~115us trip at all — the
        # controller appears thermal/chip-global with long time constants).
        pace_a = int(os.environ.get("KPACE_A", "0"))
        pace_c = int(os.environ.get("KPACE_C", "0"))
        if pace_a <= 0 and pace_c <= 0:
            return False
        a_pref = ("wq_sb", "wk_sb", "wv_sb")  # noqa: F841  (pacing off)
        n = [0]
        cnt = {"A": 0, "C": 0}

        sem_mode = os.environ.get("KPACE_MODE", "sem") == "sem"

        def noop(wait):
            n[0] += 1
            if sem_mode and wait is not None:
                # EventSemaphore re-waiting an already-satisfied DMA counter
                # threshold: ~55ns of real PE-sequencer time (RegisterMoves
                # vanish in the HW-decoded stream). DMA counters only
                # increment, so a repeated sem-ge wait can never block.
                return {
                    "debug": 0,
                    "engine": "PE",
                    "ins": [],
                    "name": f"PACE-{n[0]}",
                    "opcode": "EventSemaphore",
                    "outs": [],
                    "sync_info": {"on_update": [], "on_wait": [wait]},
                }
            return {
                "debug": 0,
                "engine": "PE",
                "ins": [{"dtype": "int32", "kind": "imm_value", "value": 0}],
                "name": f"PACE-{n[0]}",
                "opcode": "RegisterMove",
                "outs": [
                    {
                        "dtype": "int32",
                        "kind": "register_access",
                        "regref": "PE_zero",
                    }
                ],
            }

        changed = False
        for fn in d.get("functions", []):
            for bb in fn.get("blocks", []):
                out = []
                last_wait = [None]
                for inst in bb.get("instructions", []):
                    out.append(inst)
                    if inst.get("engine") != "PE":
                        continue
                    for w in (inst.get("sync_info") or {}).get("on_wait") or []:
                        if (
                            w.get("wait_mode") == "sem-ge-imm"
                            and "DMA" in (w.get("ant_name") or "")
                        ):
                            last_wait[0] = w
                    if inst["opcode"] != "Matmult":
                        continue
                    refs = [
                        i.get("memref", "")
                        for i in inst.get("ins", [])
                        if isinstance(i, dict)
                    ]
                    grp = None
                    if any(r.startswith(a_pref) for r in refs):
                        grp = ("A", pace_a)
                    elif any(r.startswith("wo_sb") for r in refs):
                        grp = ("C", pace_c)
                    if grp is None or grp[1] <= 0:
                        continue
                    cnt[grp[0]] += 1
                    if cnt[grp[0]] % grp[1] == 0:
                        out.append(noop(last_wait[0]))
                        changed = True
                bb["instructions"] = out
        return changed

    def split_multiwait(bir_json):
        d = json.loads(bir_json)
        changed = dedup_ldweights(d) if os.environ.get("KDEDUP", "1") == "1" else False
        changed |= inject_pacing(d)
        for fn in d.get("functions", []):
            for bb in fn.get("blocks", []):
                new_insts = []
                for inst in bb.get("instructions", []):
                    si = inst.get("sync_info")
                    waits = (si or {}).get("on_wait") or []
                    if len(waits) > 1:
                        changed = True
                        for w in waits[:-1]:
                            counter[0] += 1
                            new_insts.append(
                                {
                                    "debug": inst.get("debug"),
                                    "engine": inst["engine"],
                                    "ins": [],
                                    "name": f"WSPLIT-{counter[0]}",
                                    "opcode": "EventSemaphore",
                                    "outs": [],
                                    "sync_info": {"on_update": [], "on_wait": [w]},
                                }
                            )
                        si["on_wait"] = [waits[-1]]
                    new_insts.append(inst)
                bb["instructions"] = new_insts
        if not changed:
            return bir_json if isinstance(bir_json, bytes) else bir_json.encode()
        return json.dumps(d).encode()

    orig = bu.compile_bir_kernel

    def patched(bir_json, tmpdir, neff_name="file.neff"):
        return orig(split_multiwait(bir_json), tmpdir, neff_name)

    bu.compile_bir_kernel = patched
    try:
        import concourse.bass2jax as b2j

        if hasattr(b2j, "compile_bir_kernel"):
            b2j.compile_bir_kernel = patched
    except ImportError:
        pass


# ------------------------------------------------------------ kernel build
def build_nc():
    import concourse.bass as bass
    import concourse.tile as tile
    from concourse import mybir

    f32 = mybir.dt.float32
    f32r = mybir.dt.bfloat16  # value-path dtype (name kept from f32r version)
    EXP = mybir.ActivationFunctionType.Exp
    LN = mybir.ActivationFunctionType.Ln
    IDENT = mybir.ActivationFunctionType.Identity

    nc = bass.Bass()

    xt_ext = nc.dram_tensor("xt", [E, S], f32r, kind="ExternalInput")
    wq_ext = nc.dram_tensor("wq", [E, DL], f32r, kind="ExternalInput")
    wk_ext = nc.dram_tensor("wk", [E, DL], f32r, kind="ExternalInput")
    wv_ext = nc.dram_tensor("wv", [E, DL], f32r, kind="ExternalInput")
    wo_ext = nc.dram_tensor("wo", [DL, E], f32r, kind="ExternalInput")
    bo_ext = nc.dram_tensor("bo2", [E], f32, kind="ExternalInput")
    mask_ext = nc.dram_tensor("mask", [P, P], f32, kind="ExternalInput")
    vones_ext = nc.dram_tensor("vones", [P, SB * HL], f32r, kind="ExternalInput")
    bsel_ext = nc.dram_tensor("bsel", [P, P], f32r, kind="ExternalInput")
    out_ext = nc.dram_tensor("outp", [E, S], f32r, kind="ExternalOutput")

    xt_r = xt_ext.rearrange("(kt p) s -> p kt s", p=P)
    wq_r = wq_ext.rearrange("(kt p) d -> p kt d", p=P)
    wk_r = wk_ext.rearrange("(kt p) d -> p kt d", p=P)
    wv_r = wv_ext.rearrange("(kt p) d -> p kt d", p=P)
    wo_r = wo_ext.rearrange("(kt p) e -> p kt e", p=P)
    bo_r = bo_ext.rearrange("(m p) -> p m", p=P)

    with tile.TileContext(nc) as tc:
        with tc.tile_pool(name="persist", bufs=1) as pers:
            # ---- persistent SBUF tensors
            qt = [pers.tile([P, S], f32r, tag=f"qt{m}", name=f"qt{m}") for m in range(MT)]
            kt = [pers.tile([P, S], f32r, tag=f"kt{m}", name=f"kt{m}") for m in range(MT)]
            v_sb = pers.tile([P, SB, HL * VW], f32r, tag="v")
            bo_sb = pers.tile([P, MT_E], f32, tag="bo")
            mask_sb = pers.tile([P, P], f32, tag="mask")
            bsel_sb = pers.tile([P, P], f32r, tag="bsel")

            # ---- stage A: QT/KT (transposed) and V (natural) projections.
            # Chunk pairs keep SBUF under budget while letting each weight
            # tile serve 2 consecutive matmuls (Ldweights dedup elides the
            # second load).
            with (
                tc.tile_pool(name="wqkv", bufs=1) as wpool,
                tc.tile_pool(name="xt", bufs=3) as xpool,
                tc.tile_pool(name="ps_a", bufs=6, space="PSUM") as ps_a,
            ):
                wq_sb = wpool.tile([P, KT_E, DL], f32r, tag="wq")
                wk_sb = wpool.tile([P, KT_E, DL], f32r, tag="wk")
                wv_sb = wpool.tile([P, KT_E, DL], f32r, tag="wv")
                # DMA issue order matters: each queue drains FIFO, and the
                # gpsimd queue is software-DGE (~5x slower) — putting wq
                # there gated the first matmul at ~15us. wq/wk ride the two
                # hardware queues right behind the xc0 halves; only
                # late-needed tensors (wv, small stuff) go software.
                # Startup is chip-HBM-bound (8 cores pull inputs at once),
                # so only gate-critical bytes go first: the first QK matmul
                # pair needs xc0+xc1 (chunk-pair, c-inner loop) and wq's
                # m=0 slice. Everything else queues behind them.
                xt_tiles = {}
                xc0 = xpool.tile([P, KT_E, 512], f32r, tag="xt", name="xt0")
                xc1 = xpool.tile([P, KT_E, 512], f32r, tag="xt", name="xt1")
                nc.sync.dma_start(xc0[:, 0 : KT_E // 2, :], xt_r[:, 0 : KT_E // 2, 0:512])
                nc.scalar.dma_start(
                    xc0[:, KT_E // 2 :, :], xt_r[:, KT_E // 2 :, 0:512]
                )
                nc.sync.dma_start(
                    xc1[:, 0 : KT_E // 2, :], xt_r[:, 0 : KT_E // 2, 512:1024]
                )
                nc.scalar.dma_start(
                    xc1[:, KT_E // 2 :, :], xt_r[:, KT_E // 2 :, 512:1024]
                )
                xt_tiles[0] = xc0
                xt_tiles[1] = xc1
                nc.sync.dma_start(wq_sb[:, :, 0:P], wq_r[:, :, 0:P])
                nc.scalar.dma_start(wq_sb[:, :, P:DL], wq_r[:, :, P:DL])
                nc.sync.dma_start(wk_sb[:], wk_r)
                nc.gpsimd.dma_start(wv_sb[:], wv_r)

                for cp in range(SC // 2):
                    cs = (2 * cp, 2 * cp + 1)
                    xt_sb = {}
                    for c in cs:
                        if c in xt_tiles:
                            xt_sb[c] = xt_tiles[c]
                            continue
                        xc = xpool.tile(
                            [P, KT_E, 512], f32r, tag="xt", name=f"xt{c}"
                        )
                        nc.sync.dma_start(
                            xc[:], xt_r[:, :, 512 * c : 512 * (c + 1)]
                        )
                        xt_sb[c] = xc

                    for w_sb, dst in ((wq_sb, qt), (wk_sb, kt)):
                        for m in range(MT):
                            pss = {
                                c: ps_a.tile(
                                    [P, 512], f32, tag="ps_a", name=f"psa{m}_{c}"
                                )
                                for c in cs
                            }
                            if cp == 0 and m == 0 and w_sb is wq_sb:
                                # c-solo passes: the very first matmul then
                                # gates on xc0 + wq's m0 slice only (~1.25MB
                                # of HBM-contended startup DMA, not 2.25MB);
                                # costs one extra weight pass (8 LDWs)
                                for c in cs:
                                    for k in range(KT_E):
                                        nc.tensor.matmul(
                                            pss[c][:],
                                            w_sb[:, k, P * m : P * (m + 1)],
                                            xt_sb[c][:, k, :],
                                            start=(k == 0),
                                            stop=(k == KT_E - 1),
                                        )
                            else:
                                for k in range(KT_E):
                                    for c in cs:
                                        nc.tensor.matmul(
                                            pss[c][:],
                                            w_sb[:, k, P * m : P * (m + 1)],
                                            xt_sb[c][:, k, :],
                                            start=(k == 0),
                                            stop=(k == KT_E - 1),
                                        )
                            for c in cs:
                                nc.vector.tensor_copy(
                                    dst[m][:, 512 * c : 512 * (c + 1)], pss[c][:]
                                )

                    # V natural: psum[sb] = sum_k XT[k, sblock].T @ WV[k, :]
                    for c in cs:
                        for s in range(4):
                            sb = 4 * c + s
                            ps = ps_a.tile(
                                [P, 512], f32, tag="ps_a", name=f"psv{sb}"
                            )
                            for k in range(KT_E):
                                nc.tensor.matmul(
                                    ps[:],
                                    xt_sb[c][:, k, P * s : P * (s + 1)],
                                    wv_sb[:, k, :],
                                    start=(k == 0),
                                    stop=(k == KT_E - 1),
                                )
                            for h in range(HL):
                                nc.vector.tensor_copy(
                                    v_sb[:, sb, VW * h : VW * h + DH],
                                    ps[:, DH * h : DH * (h + 1)],
                                )

            # small/strided loads issued after the stage-A streams so they
            # don't delay the HWDGE queues feeding the first matmuls
            nc.gpsimd.dma_start(bo_sb[:], bo_r)
            nc.gpsimd.dma_start(mask_sb[:], mask_ext[:, :])
            nc.gpsimd.dma_start(bsel_sb[:], bsel_ext[:])
            # ones column for the fused row-sum in the AV matmul (DMA'd in:
            # memset can't write 1.0 into a bf16 tensor on this compiler)
            ones_col = v_sb[:].rearrange("p sb (h c) -> p sb h c", c=VW)[
                :, :, :, DH : DH + 1
            ]
            nc.sync.dma_start(
                ones_col, vones_ext.rearrange("p (sb h one) -> p sb h one", h=HL, one=1)
            )

            # ---- stage B: block-causal attention in transposed layout
            with tc.tile_pool(name="late", bufs=1) as late:
                # allocated after the stage-A pools are gone (SBUF budget);
                # Wo prefetches while attention runs
                ct = [
                    late.tile([P, KT_D, 512], f32r, tag=f"ct{i}", name=f"ct{i}")
                    for i in range(SC)
                ]
                wo_sb = late.tile([P, KT_D, E], f32r, tag="wo")
                nc.sync.dma_start(wo_sb[:], wo_r)
                _stage_b(
                    nc, tc, qt, kt, v_sb, ct, mask_sb, bsel_sb, EXP, LN, f32, f32r
                )

                # ---- stage C: out^T = WoT-slice.T @ ctx^T (+ bias, group 0)
                with (
                    tc.tile_pool(name="ostage", bufs=4) as opool,
                    tc.tile_pool(name="ps_p", bufs=8, space="PSUM") as ps_p,
                ):
                    for m in range(MT_E):
                        pss = [
                            ps_p.tile([P, 512], f32, tag="ps_p", name=f"psp{m}_{s}")
                            for s in range(SC)
                        ]
                        for k in range(KT_D):
                            for s in range(SC):
                                nc.tensor.matmul(
                                    pss[s][:],
                                    wo_sb[:, k, P * m : P * (m + 1)],
                                    ct[s][:, k, :],
                                    start=(k == 0),
                                    stop=(k == KT_D - 1),
                                )
                        for s in range(SC):
                            ot = opool.tile(
                                [P, 512], f32r, tag="ostage", name=f"ot{m}_{s}"
                            )
                            nc.scalar.activation(
                                ot[:], pss[s][:], IDENT, bias=bo_sb[:, m : m + 1]
                            )
                            nc.sync.dma_start(
                                out_ext[
                                    P * m : P * (m + 1), 512 * s : 512 * (s + 1)
                                ],
                                ot[:],
                            )

    return nc


def _stage_b(nc, tc, qt, kt, v_sb, ct, mask_sb, bsel_sb, EXP, LN, f32, f32r):
    with (
        tc.tile_pool(name="probs", bufs=10) as ppool,
        tc.tile_pool(name="fin", bufs=2) as fpool,
        tc.tile_pool(name="ps_s", bufs=3, space="PSUM") as ps_s,
        tc.tile_pool(name="ps_ctx", bufs=3, space="PSUM") as ps_ctx,
        tc.tile_pool(name="ps_bc", bufs=2, space="PSUM") as ps_bc,
    ):
        for c in range(SC):
            q_lo, q_hi = 512 * c, 512 * (c + 1)
            # row sums at 32-aligned partitions {0,32} x 4 free slots so the
            # broadcast matmul can read the reciprocals in place (rhs/lhsT
            # base partition must be 0/32/64)
            sums_sb = fpool.tile([P, 1024], f32, tag="sums", name=f"sums{c}")
            if c < 2:
                # first use of each ring slot: clear garbage so the unused
                # lanes of Ln/Exp stay finite (the K=33 broadcast matmul
                # multiplies them by zero, but 0*NaN would poison it)
                nc.vector.memset(sums_sb[:], 1.0)
            for t in range(MT):  # head pair (2t, 2t+1)
                heads = (2 * t, 2 * t + 1)
                nblk = 4 * c + 4
                ctx_ps = {}
                probs = {}
                for h in heads:
                    ctx_ps[h] = ps_ctx.tile([VW, 512], f32, tag="ps_ctx", name=f"ctx_{c}_{h}")

                def av_mm(h, j, first, last):
                    r = j - 4 * c
                    lo = P * r if r > 0 else 0
                    nc.tensor.matmul(
                        ctx_ps[h][:, lo:512],
                        v_sb[:, j, VW * h : VW * (h + 1)],
                        probs[(h, j)][:, lo:512],
                        start=first,
                        stop=last,
                    )

                for j in range(nblk):
                    r = j - 4 * c  # >=0 only on diagonal band
                    for h in heads:
                        hp = DH * (h % 2)
                        ps = ps_s.tile([P, 512], f32, tag="ps_s")
                        # scoresT[kv_block j, q chunk c] = K_j @ Q^T; on the
                        # diagonal band only cols >= 128r are causally live.
                        # bf16 has no small-N rate penalty, so trim all of it.
                        slo = P * r if r > 0 else 0
                        nc.tensor.matmul(
                            ps[:, slo:512],
                            kt[t][hp : hp + DH, P * j : P * (j + 1)],
                            qt[t][hp : hp + DH, q_lo + slo : q_hi],
                            start=True,
                            stop=True,
                        )
                        if r >= 0:
                            # triangle mask on the diagonal 128x128
                            nc.vector.tensor_add(
                                ps[:, P * r : P * (r + 1)],
                                ps[:, P * r : P * (r + 1)],
                                mask_sb[:],
                            )
                        pr = ppool.tile(
                            [P, 512], f32r, tag="probs", name=f"pr_{c}_{h}_{j}"
                        )
                        probs[(h, j)] = pr
                        lo = P * r if r > 0 else 0
                        nc.scalar.activation(
                            pr[:, lo:512], ps[:, lo:512], EXP, scale=SCALE
                        )
                    if j >= 1:
                        for h in heads:
                            av_mm(h, j - 1, first=(j == 1), last=False)
                for h in heads:
                    av_mm(h, nblk - 1, first=(nblk == 1), last=True)

                # stash unnormalized ctx + row sums
                for h in heads:
                    hp = DH * (h % 2)
                    nc.vector.tensor_copy(
                        ct[c][hp : hp + DH, t, :], ctx_ps[h][0:DH, :]
                    )
                    nc.vector.tensor_copy(
                        sums_sb[
                            64 * (t % 2) + 32 * (h % 2) : 64 * (t % 2) + 32 * (h % 2) + 1,
                            512 * (t // 2) : 512 * (t // 2) + 512,
                        ],
                        ctx_ps[h][DH : DH + 1, :],
                    )

            # ---- batched softmax normalization for this chunk: 1/sums as
            # exp(-ln(sums)) on ACT (the DVE-reciprocal variant measured
            # slower overall), written as bf16 directly where the broadcast
            # matmul reads it. bc matmuls are ordered by sums-row parity so
            # the all-ones stationary reloads dedup to one per row.
            lns = fpool.tile([P, 1024], f32, tag="lns", name=f"lns{c}", bufs=1)
            nc.scalar.activation(lns[:], sums_sb[:], LN)
            recs = fpool.tile([P, 1024], f32r, tag="recs", name=f"recs{c}")
            nc.scalar.activation(recs[:], lns[:], EXP, scale=-1.0)
            # one K=2 matmul per head PAIR: the block-indicator stationary
            # (bsel rows p%2==0 -> ones|zeros, p%2==1 -> zeros|ones) routes
            # rec row h_even to out partitions 0-63 and h_odd to 64-127, so
            # a single [128,512] psum serves both heads and one DVE mul
            # normalizes the whole pair slice. t-order (0,2,1,3) makes the
            # two bsel loads per base dedup.
            for t in (0, 2, 1, 3):
                rp = 64 * (t % 2)
                bc = ps_bc.tile([P, 512], f32, tag="ps_bc", name=f"bc{c}_{t}")
                nc.tensor.matmul(
                    bc[:],
                    bsel_sb[rp : rp + 33, :],
                    recs[rp : rp + 33, 512 * (t // 2) : 512 * (t // 2) + 512],
                    start=True,
                    stop=True,
                )
                nc.vector.tensor_mul(
                    ct[c][:, t, :],
                    ct[c][:, t, :],
                    bc[:],
                )


# ------------------------------------------------------------ PJRT runner
class _Runner:
    """Compile once, run many: mirrors bass2jax.run_bass_via_pjrt with a
    cached jitted executable."""

    def __init__(self, nc):
        import jax
        import jax.numpy  # noqa: F401
        from jax.sharding import Mesh, PartitionSpec
        from jax.experimental.shard_map import shard_map
        import concourse.bass2jax as b2j
        from concourse import mybir

        b2j.install_neuronx_cc_hook()
        self.jax = jax
        partition_name = (
            nc.partition_id_tensor.name if nc.partition_id_tensor else None
        )
        in_names = []
        out_names = []
        out_avals = []
        self.zero_shapes = []
        for alloc in nc.m.functions[0].allocations:
            if not isinstance(alloc, mybir.MemoryLocationSet):
                continue
            name = alloc.memorylocations[0].name
            if alloc.kind == "ExternalInput":
                if name == partition_name:
                    continue
                in_names.append(name)
            elif alloc.kind == "ExternalOutput":
                shape = tuple(alloc.tensor_shape)
                dtype = mybir.dt.np(alloc.dtype)
                out_names.append(name)
                out_avals.append(jax.core.ShapedArray(shape, dtype))
                self.zero_shapes.append((shape, dtype))
        self.in_names = in_names
        self.out_names = out_names
        self.out_avals = out_avals
        n_params = len(in_names)
        n_outs = len(out_avals)
        all_in = list(in_names) + list(out_names)
        if partition_name is not None:
            all_in.append(partition_name)

        def _body(*args):
            operands = list(args)
            if partition_name is not None:
                operands.append(b2j.partition_id_tensor())
            outs = b2j._bass_exec_p.bind(
                *operands,
                out_avals=tuple(out_avals),
                in_names=tuple(all_in),
                out_names=tuple(out_names),
                lowering_input_output_aliases=(),
                sim_require_finite=True,
                sim_require_nnan=True,
                nc=nc,
            )
            return tuple(outs)

        devices = jax.devices()[:NCORES]
        assert len(devices) == NCORES, f"need {NCORES} cores, got {len(devices)}"
        self.mesh = Mesh(np.asarray(devices), ("core",))
        in_specs = (PartitionSpec("core"),) * (n_params + n_outs)
        out_specs = (PartitionSpec("core"),) * n_outs
        self.fn = jax.jit(
            shard_map(
                _body,
                mesh=self.mesh,
                in_specs=in_specs,
                out_specs=out_specs,
                check_rep=False,
            ),
            donate_argnums=tuple(range(n_params, n_params + n_outs)),
            keep_unused=True,
        )

    def run(self, in_maps):
        concat_in = [
            np.concatenate([np.asarray(m[name]) for m in in_maps], axis=0)
            for name in self.in_names
        ]
        zeros = [
            np.zeros((NCORES * s[0], *s[1:]), dt) for s, dt in self.zero_shapes
        ]
        outs = self.fn(*concat_in, *zeros)
        return [
            {
                name: np.asarray(outs[i]).reshape(
                    NCORES, *self.out_avals[i].shape
                )[c]
                for i, name in enumerate(self.out_names)
            }
            for c in range(NCORES)
        ]


_cache = {}


def _get_runner():
    if "runner" not in _cache:
        _install_syncfix()
        _cache["runner"] = _Runner(build_nc())
    return _cache["runner"]


def make_in_maps(X, Wq, Wk, Wv, Wo, bo):
    import ml_dtypes

    bf16 = ml_dtypes.bfloat16
    X = np.asarray(X, dtype=np.float32)
    Wq = np.asarray(Wq, dtype=np.float32)
    Wk = np.asarray(Wk, dtype=np.float32)
    Wv = np.asarray(Wv, dtype=np.float32)
    Wo = np.asarray(Wo, dtype=np.float32)
    bo = np.asarray(bo, dtype=np.float32)

    kv = np.arange(P)[:, None]
    qq = np.arange(P)[None, :]
    mask = np.where(kv > qq, np.float32(NEG), np.float32(0.0))

    in_maps = []
    for core in range(NCORES):
        b, g = divmod(core, 2)
        h0 = HL * g
        in_maps.append(
            {
                "xt": np.ascontiguousarray(X[b].T.astype(bf16)),
                "wq": np.ascontiguousarray(
                    Wq[h0 : h0 + HL].transpose(1, 0, 2).reshape(E, DL).astype(bf16)
                ),
                "wk": np.ascontiguousarray(
                    Wk[h0 : h0 + HL].transpose(1, 0, 2).reshape(E, DL).astype(bf16)
                ),
                "wv": np.ascontiguousarray(
                    Wv[h0 : h0 + HL].transpose(1, 0, 2).reshape(E, DL).astype(bf16)
                ),
                "wo": np.ascontiguousarray(Wo[:, DL * g : DL * (g + 1)].T.astype(bf16)),
                "bo2": bo if g == 0 else np.zeros_like(bo),
                "mask": mask,
                "vones": np.ones((P, SB * HL), dtype=bf16),
                "o64": np.ones((P, DH), dtype=bf16),
                "bsel": np.stack(
                    [
                        np.concatenate([np.ones(DH), np.zeros(DH)])
                        if p in (0, 64)
                        else np.concatenate([np.zeros(DH), np.ones(DH)])
                        if p in (32, 96)
                        else np.zeros(P)
                        for p in range(P)
                    ]
                ).astype(bf16),
            }
        )
    return in_maps


def assemble(results):
    out = np.empty((B, S, E), dtype=np.float32)
    for b in range(B):
        acc = np.asarray(results[2 * b]["outp"], dtype=np.float32) + np.asarray(
            results[2 * b + 1]["outp"], dtype=np.float32
        )
        out[b] = acc.T
    return out


def kernel(X, Wq, Wk, Wv, Wo, bo):
    runner = _get_runner()
    in_maps = make_in_maps(X, Wq, Wk, Wv, Wo, bo)
    results = runner.run(in_maps)
    return assemble(results)


# revision 46
# speedup vs baseline: 1.1025x; 1.0193x over previous
"""Multi-head causal attention (B=4, S=2048, E=1024, H=16, Dh=64) on 8
Trainium2 NeuronCores.

Sharding: data-parallel over the 4 batch elements x tensor-parallel over
heads (2 groups of 8). Core 2b+g handles batch b, heads 8g..8g+7. Each core
computes Q^T/K^T (head dim on partitions), V (natural layout, with a fused
ones-column so the attention-weight row sums fall out of the same matmul),
block-causal scores in transposed [kv, q] layout (so no transposes are ever
needed: softmax normalization is a reciprocal + partition-broadcast), the
local-head context, and the output projection against its slice of Wo. The
two partial projections per batch are summed on the host (the TP
"all-reduce" of the sharding hint, done at gather time), which also absorbs
the out-transpose: the kernel emits out^T [E, S].

All matmuls run in bfloat16 (fp32 accumulate): same PE rate as float32r but
half the SBUF/DMA/LDWEIGHTS traffic and lower PE power draw. Power draw
matters directly here: the chip's activity throttle (HAM util cap, k=4/8)
engages after ~90us of sustained full-width matmul and halves PE duty. The
f32r version lost ~170us to it; bf16 loses ~60us. Stage B's instruction mix
(64-contraction scores, per-matmul LDWEIGHTS gaps) keeps it just under the
throttle's release threshold — deliberately denser variants (fused exps,
eager LDW dedup inside stage B) measured SLOWER because the throttle then
never releases. Hence LDWEIGHTS dedup only fires where the natural
instruction order creates back-to-back duplicates (stages A and C).
"""

import json
import os
import sys

for _p in ("/opt/trn_rl_repo",):
    if _p not in sys.path:
        sys.path.insert(0, _p)

import numpy as np

# ---------------------------------------------------------------- constants
B = 4
S = 2048
E = 1024
H = 16
DH = 64
HL = 8  # heads per core
DL = HL * DH  # 512, local head dim
P = 128
NCORES = 8
SCALE = 1.0 / 8.0  # 1/sqrt(DH)
NEG = -1.0e30

KT_E = E // P  # 8  k-tiles over embed dim
MT = DL // P  # 4  m-tiles over local head dim (2 heads per m-tile)
SC = S // 512  # 4  512-wide chunks over sequence
SB = S // P  # 16 128-blocks over sequence
KT_D = DL // P  # 4  k-tiles over local head dim (proj contraction)
MT_E = E // P  # 8  m-tiles over embed dim (proj output)
VW = DH + 1  # 65: V columns per head + ones column


# ------------------------------------------------- BIR post-processing
# 1. Ldweights dedup: bass splits every bf16 matmul into Ldweights+Matmult
#    (walrus ldw-opt rejects these pairs, so the elision happens here).
#    Only back-to-back identical loads are elided, which by construction
#    only exist in stages A and C — stage B's per-matmul loads are kept as
#    deliberate pacing (see module docstring).
# 2. Multi-wait splitting: this walrus accepts one sync-wait per
#    instruction; extras become single-wait EventSemaphores.
_syncfix_done = [False]


def _install_syncfix():
    if _syncfix_done[0]:
        return
    _syncfix_done[0] = True
    import concourse.bass_utils as bu

    counter = [0]

    def dedup_ldweights(d):
        changed = False
        for fn in d.get("functions", []):
            for bb in fn.get("blocks", []):
                last_sig = None
                pending = []
                out = []
                for inst in bb.get("instructions", []):
                    if inst.get("engine") != "PE":
                        out.append(inst)
                        continue
                    op = inst["opcode"]
                    if op == "Ldweights":
                        sig = json.dumps(
                            [
                                inst.get("ins"),
                                inst.get("tile_position"),
                                inst.get("tile_size"),
                            ],
                            sort_keys=True,
                        )
                        if sig == last_sig and not (
                            (inst.get("sync_info") or {}).get("on_update")
                        ):
                            changed = True
                            w = (inst.get("sync_info") or {}).get("on_wait") or []
                            pending.extend(w)
                            continue
                        last_sig = sig
                    elif op in ("Matmult", "EventSemaphore"):
                        pass  # PE weight state preserved
                    else:
                        last_sig = None
                    if pending:
                        si = inst.setdefault(
                            "sync_info", {"on_update": [], "on_wait": []}
                        )
                        si["on_wait"] = list(si.get("on_wait") or []) + pending
                        pending = []
                    out.append(inst)
                bb["instructions"] = out
        return changed

    def inject_pacing(d):
        """Insert idempotent PE no-ops (RegisterMove PE_zero<-0) after every
        Nth full-width projection matmul. Stages A and C run 128-contraction
        matmuls back to back at ~100% PE-array duty, which charges the
        chip's power throttle and costs ~50% duty caps that bleed far into
        stage B. A ~10% duty gap keeps the integrator under its trip point.
        Stage-B matmuls (64-contraction, naturally padded by LDWEIGHTS) are
        left untouched."""
        # Pacing is OFF by default: measured no effect on the throttle trip
        # point from either mechanism (RegisterMoves vanish in the
        # HW-decoded stream; EventSemaphore re-waits cost ~55ns each but the
        # ~11% duty dilution did not shift the ~115us trip at all — the
        # controller appears thermal/chip-global with long time constants).
        pace_a = int(os.environ.get("KPACE_A", "0"))
        pace_c = int(os.environ.get("KPACE_C", "0"))
        if pace_a <= 0 and pace_c <= 0:
            return False
        a_pref = ("wq_sb", "wk_sb", "wv_sb")  # noqa: F841  (pacing off)
        n = [0]
        cnt = {"A": 0, "C": 0}

        sem_mode = os.environ.get("KPACE_MODE", "sem") == "sem"

        def noop(wait):
            n[0] += 1
            if sem_mode and wait is not None:
                # EventSemaphore re-waiting an already-satisfied DMA counter
                # threshold: ~55ns of real PE-sequencer time (RegisterMoves
                # vanish in the HW-decoded stream). DMA counters only
                # increment, so a repeated sem-ge wait can never block.
                return {
                    "debug": 0,
                    "engine": "PE",
                    "ins": [],
                    "name": f"PACE-{n[0]}",
                    "opcode": "EventSemaphore",
                    "outs": [],
                    "sync_info": {"on_update": [], "on_wait": [wait]},
                }
            return {
                "debug": 0,
                "engine": "PE",
                "ins": [{"dtype": "int32", "kind": "imm_value", "value": 0}],
                "name": f"PACE-{n[0]}",
                "opcode": "RegisterMove",
                "outs": [
                    {
                        "dtype": "int32",
                        "kind": "register_access",
                        "regref": "PE_zero",
                    }
                ],
            }

        changed = False
        for fn in d.get("functions", []):
            for bb in fn.get("blocks", []):
                out = []
                last_wait = [None]
                for inst in bb.get("instructions", []):
                    out.append(inst)
                    if inst.get("engine") != "PE":
                        continue
                    for w in (inst.get("sync_info") or {}).get("on_wait") or []:
                        if (
                            w.get("wait_mode") == "sem-ge-imm"
                            and "DMA" in (w.get("ant_name") or "")
                        ):
                            last_wait[0] = w
                    if inst["opcode"] != "Matmult":
                        continue
                    refs = [
                        i.get("memref", "")
                        for i in inst.get("ins", [])
                        if isinstance(i, dict)
                    ]
                    grp = None
                    if any(r.startswith(a_pref) for r in refs):
                        grp = ("A", pace_a)
                    elif any(r.startswith("wo_sb") for r in refs):
                        grp = ("C", pace_c)
                    if grp is None or grp[1] <= 0:
                        continue
                    cnt[grp[0]] += 1
                    if cnt[grp[0]] % grp[1] == 0:
                        out.append(noop(last_wait[0]))
                        changed = True
                bb["instructions"] = out
        return changed

    def split_multiwait(bir_json):
        d = json.loads(bir_json)
        changed = dedup_ldweights(d) if os.environ.get("KDEDUP", "1") == "1" else False
        changed |= inject_pacing(d)
        for fn in d.get("functions", []):
            for bb in fn.get("blocks", []):
                new_insts = []
                for inst in bb.get("instructions", []):
                    si = inst.get("sync_info")
                    waits = (si or {}).get("on_wait") or []
                    if len(waits) > 1:
                        changed = True
                        for w in waits[:-1]:
                            counter[0] += 1
                            new_insts.append(
                                {
                                    "debug": inst.get("debug"),
                                    "engine": inst["engine"],
                                    "ins": [],
                                    "name": f"WSPLIT-{counter[0]}",
                                    "opcode": "EventSemaphore",
                                    "outs": [],
                                    "sync_info": {"on_update": [], "on_wait": [w]},
                                }
                            )
                        si["on_wait"] = [waits[-1]]
                    new_insts.append(inst)
                bb["instructions"] = new_insts
        if not changed:
            return bir_json if isinstance(bir_json, bytes) else bir_json.encode()
        return json.dumps(d).encode()

    orig = bu.compile_bir_kernel

    def patched(bir_json, tmpdir, neff_name="file.neff"):
        return orig(split_multiwait(bir_json), tmpdir, neff_name)

    bu.compile_bir_kernel = patched
    try:
        import concourse.bass2jax as b2j

        if hasattr(b2j, "compile_bir_kernel"):
            b2j.compile_bir_kernel = patched
    except ImportError:
        pass


# ------------------------------------------------------------ kernel build
def build_nc():
    import concourse.bass as bass
    import concourse.tile as tile
    from concourse import mybir

    f32 = mybir.dt.float32
    f32r = mybir.dt.bfloat16  # value-path dtype (name kept from f32r version)
    EXP = mybir.ActivationFunctionType.Exp
    LN = mybir.ActivationFunctionType.Ln
    IDENT = mybir.ActivationFunctionType.Identity

    nc = bass.Bass()

    xt_ext = nc.dram_tensor("xt", [E, S], f32r, kind="ExternalInput")
    wq_ext = nc.dram_tensor("wq", [E, DL], f32r, kind="ExternalInput")
    wk_ext = nc.dram_tensor("wk", [E, DL], f32r, kind="ExternalInput")
    wv_ext = nc.dram_tensor("wv", [E, DL], f32r, kind="ExternalInput")
    wo_ext = nc.dram_tensor("wo", [DL, E], f32r, kind="ExternalInput")
    bo_ext = nc.dram_tensor("bo2", [E], f32, kind="ExternalInput")
    mask_ext = nc.dram_tensor("mask", [P, P], f32, kind="ExternalInput")
    vones_ext = nc.dram_tensor("vones", [P, SB * HL], f32r, kind="ExternalInput")
    bsel_ext = nc.dram_tensor("bsel", [P, P], f32r, kind="ExternalInput")
    out_ext = nc.dram_tensor("outp", [E, S], f32r, kind="ExternalOutput")

    xt_r = xt_ext.rearrange("(kt p) s -> p kt s", p=P)
    wq_r = wq_ext.rearrange("(kt p) d -> p kt d", p=P)
    wk_r = wk_ext.rearrange("(kt p) d -> p kt d", p=P)
    wv_r = wv_ext.rearrange("(kt p) d -> p kt d", p=P)
    wo_r = wo_ext.rearrange("(kt p) e -> p kt e", p=P)
    bo_r = bo_ext.rearrange("(m p) -> p m", p=P)

    with tile.TileContext(nc) as tc:
        with tc.tile_pool(name="persist", bufs=1) as pers:
            # ---- persistent SBUF tensors
            qt = [pers.tile([P, S], f32r, tag=f"qt{m}", name=f"qt{m}") for m in range(MT)]
            kt = [pers.tile([P, S], f32r, tag=f"kt{m}", name=f"kt{m}") for m in range(MT)]
            v_sb = pers.tile([P, SB, HL * VW], f32r, tag="v")
            bo_sb = pers.tile([P, MT_E], f32, tag="bo")
            mask_sb = pers.tile([P, P], f32, tag="mask")
            bsel_sb = pers.tile([P, P], f32r, tag="bsel")

            # ---- stage A: QT/KT (transposed) and V (natural) projections.
            # Chunk pairs keep SBUF under budget while letting each weight
            # tile serve 2 consecutive matmuls (Ldweights dedup elides the
            # second load).
            with (
                tc.tile_pool(name="wqkv", bufs=1) as wpool,
                tc.tile_pool(name="xt", bufs=3) as xpool,
                tc.tile_pool(name="ps_a", bufs=6, space="PSUM") as ps_a,
            ):
                wq_sb = wpool.tile([P, KT_E, DL], f32r, tag="wq")
                wk_sb = wpool.tile([P, KT_E, DL], f32r, tag="wk")
                wv_sb = wpool.tile([P, KT_E, DL], f32r, tag="wv")
                # DMA issue order matters: each queue drains FIFO, and the
                # gpsimd queue is software-DGE (~5x slower) — putting wq
                # there gated the first matmul at ~15us. wq/wk ride the two
                # hardware queues right behind the xc0 halves; only
                # late-needed tensors (wv, small stuff) go software.
                # Startup is chip-HBM-bound (8 cores pull inputs at once),
                # so only gate-critical bytes go first: the first QK matmul
                # pair needs xc0+xc1 (chunk-pair, c-inner loop) and wq's
                # m=0 slice. Everything else queues behind them.
                xt_tiles = {}
                xc0 = xpool.tile([P, KT_E, 512], f32r, tag="xt", name="xt0")
                xc1 = xpool.tile([P, KT_E, 512], f32r, tag="xt", name="xt1")
                # exact gate of the very first matmul chain (m=0 c=0 solo
                # pass): xc0 + wq's m0 slice — those ride FIRST on both HW
                # queues; xc1 (needed ~3.4us later, covered by the c=0
                # accumulation) queues behind.
                nc.sync.dma_start(xc0[:, 0 : KT_E // 2, :], xt_r[:, 0 : KT_E // 2, 0:512])
                nc.scalar.dma_start(
                    xc0[:, KT_E // 2 :, :], xt_r[:, KT_E // 2 :, 0:512]
                )
                nc.sync.dma_start(wq_sb[:, :, 0:P], wq_r[:, :, 0:P])
                nc.scalar.dma_start(
                    xc1[:, KT_E // 2 :, :], xt_r[:, KT_E // 2 :, 512:1024]
                )
                nc.sync.dma_start(
                    xc1[:, 0 : KT_E // 2, :], xt_r[:, 0 : KT_E // 2, 512:1024]
                )
                xt_tiles[0] = xc0
                xt_tiles[1] = xc1
                nc.scalar.dma_start(wq_sb[:, :, P:DL], wq_r[:, :, P:DL])
                nc.sync.dma_start(wk_sb[:], wk_r)
                nc.gpsimd.dma_start(wv_sb[:], wv_r)

                for cp in range(SC // 2):
                    cs = (2 * cp, 2 * cp + 1)
                    xt_sb = {}
                    for c in cs:
                        if c in xt_tiles:
                            xt_sb[c] = xt_tiles[c]
                            continue
                        xc = xpool.tile(
                            [P, KT_E, 512], f32r, tag="xt", name=f"xt{c}"
                        )
                        nc.sync.dma_start(
                            xc[:], xt_r[:, :, 512 * c : 512 * (c + 1)]
                        )
                        xt_sb[c] = xc

                    for w_sb, dst in ((wq_sb, qt), (wk_sb, kt)):
                        for m in range(MT):
                            pss = {
                                c: ps_a.tile(
                                    [P, 512], f32, tag="ps_a", name=f"psa{m}_{c}"
                                )
                                for c in cs
                            }
                            if cp == 0 and m == 0 and w_sb is wq_sb:
                                # c-solo passes: the very first matmul then
                                # gates on xc0 + wq's m0 slice only (~1.25MB
                                # of HBM-contended startup DMA, not 2.25MB);
                                # costs one extra weight pass (8 LDWs)
                                for c in cs:
                                    for k in range(KT_E):
                                        nc.tensor.matmul(
                                            pss[c][:],
                                            w_sb[:, k, P * m : P * (m + 1)],
                                            xt_sb[c][:, k, :],
                                            start=(k == 0),
                                            stop=(k == KT_E - 1),
                                        )
                            else:
                                for k in range(KT_E):
                                    for c in cs:
                                        nc.tensor.matmul(
                                            pss[c][:],
                                            w_sb[:, k, P * m : P * (m + 1)],
                                            xt_sb[c][:, k, :],
                                            start=(k == 0),
                                            stop=(k == KT_E - 1),
                                        )
                            for c in cs:
                                nc.vector.tensor_copy(
                                    dst[m][:, 512 * c : 512 * (c + 1)], pss[c][:]
                                )

                    # V natural: psum[sb] = sum_k XT[k, sblock].T @ WV[k, :]
                    for c in cs:
                        for s in range(4):
                            sb = 4 * c + s
                            ps = ps_a.tile(
                                [P, 512], f32, tag="ps_a", name=f"psv{sb}"
                            )
                            for k in range(KT_E):
                                nc.tensor.matmul(
                                    ps[:],
                                    xt_sb[c][:, k, P * s : P * (s + 1)],
                                    wv_sb[:, k, :],
                                    start=(k == 0),
                                    stop=(k == KT_E - 1),
                                )
                            for h in range(HL):
                                nc.vector.tensor_copy(
                                    v_sb[:, sb, VW * h : VW * h + DH],
                                    ps[:, DH * h : DH * (h + 1)],
                                )

            # small/strided loads issued after the stage-A streams so they
            # don't delay the HWDGE queues feeding the first matmuls
            nc.gpsimd.dma_start(bo_sb[:], bo_r)
            nc.gpsimd.dma_start(mask_sb[:], mask_ext[:, :])
            nc.gpsimd.dma_start(bsel_sb[:], bsel_ext[:])
            # ones column for the fused row-sum in the AV matmul (DMA'd in:
            # memset can't write 1.0 into a bf16 tensor on this compiler)
            ones_col = v_sb[:].rearrange("p sb (h c) -> p sb h c", c=VW)[
                :, :, :, DH : DH + 1
            ]
            nc.sync.dma_start(
                ones_col, vones_ext.rearrange("p (sb h one) -> p sb h one", h=HL, one=1)
            )

            # ---- stage B: block-causal attention in transposed layout
            with tc.tile_pool(name="late", bufs=1) as late:
                # allocated after the stage-A pools are gone (SBUF budget);
                # Wo prefetches while attention runs
                ct = [
                    late.tile([P, KT_D, 512], f32r, tag=f"ct{i}", name=f"ct{i}")
                    for i in range(SC)
                ]
                wo_sb = late.tile([P, KT_D, E], f32r, tag="wo")
                nc.sync.dma_start(wo_sb[:], wo_r)
                _stage_b(
                    nc, tc, qt, kt, v_sb, ct, mask_sb, bsel_sb, EXP, LN, f32, f32r
                )

                # ---- stage C: out^T = WoT-slice.T @ ctx^T (+ bias, group 0)
                with (
                    tc.tile_pool(name="ostage", bufs=4) as opool,
                    tc.tile_pool(name="ps_p", bufs=8, space="PSUM") as ps_p,
                ):
                    for m in range(MT_E):
                        pss = [
                            ps_p.tile([P, 512], f32, tag="ps_p", name=f"psp{m}_{s}")
                            for s in range(SC)
                        ]
                        for k in range(KT_D):
                            for s in range(SC):
                                nc.tensor.matmul(
                                    pss[s][:],
                                    wo_sb[:, k, P * m : P * (m + 1)],
                                    ct[s][:, k, :],
                                    start=(k == 0),
                                    stop=(k == KT_D - 1),
                                )
                        for s in range(SC):
                            ot = opool.tile(
                                [P, 512], f32r, tag="ostage", name=f"ot{m}_{s}"
                            )
                            nc.scalar.activation(
                                ot[:], pss[s][:], IDENT, bias=bo_sb[:, m : m + 1]
                            )
                            nc.sync.dma_start(
                                out_ext[
                                    P * m : P * (m + 1), 512 * s : 512 * (s + 1)
                                ],
                                ot[:],
                            )

    return nc


def _stage_b(nc, tc, qt, kt, v_sb, ct, mask_sb, bsel_sb, EXP, LN, f32, f32r):
    with (
        tc.tile_pool(name="probs", bufs=10) as ppool,
        tc.tile_pool(name="fin", bufs=2) as fpool,
        tc.tile_pool(name="ps_s", bufs=3, space="PSUM") as ps_s,
        tc.tile_pool(name="ps_ctx", bufs=3, space="PSUM") as ps_ctx,
        tc.tile_pool(name="ps_bc", bufs=2, space="PSUM") as ps_bc,
    ):
        for c in range(SC):
            q_lo, q_hi = 512 * c, 512 * (c + 1)
            # row sums at 32-aligned partitions {0,32} x 4 free slots so the
            # broadcast matmul can read the reciprocals in place (rhs/lhsT
            # base partition must be 0/32/64)
            sums_sb = fpool.tile([P, 1024], f32, tag="sums", name=f"sums{c}")
            if c < 2:
                # first use of each ring slot: clear garbage so the unused
                # lanes of Ln/Exp stay finite (the K=33 broadcast matmul
                # multiplies them by zero, but 0*NaN would poison it)
                nc.vector.memset(sums_sb[:], 1.0)
            for t in range(MT):  # head pair (2t, 2t+1)
                heads = (2 * t, 2 * t + 1)
                nblk = 4 * c + 4
                ctx_ps = {}
                probs = {}
                for h in heads:
                    ctx_ps[h] = ps_ctx.tile([VW, 512], f32, tag="ps_ctx", name=f"ctx_{c}_{h}")

                def av_mm(h, j, first, last):
                    r = j - 4 * c
                    lo = P * r if r > 0 else 0
                    nc.tensor.matmul(
                        ctx_ps[h][:, lo:512],
                        v_sb[:, j, VW * h : VW * (h + 1)],
                        probs[(h, j)][:, lo:512],
                        start=first,
                        stop=last,
                    )

                for j in range(nblk):
                    r = j - 4 * c  # >=0 only on diagonal band
                    for h in heads:
                        hp = DH * (h % 2)
                        ps = ps_s.tile([P, 512], f32, tag="ps_s")
                        # scoresT[kv_block j, q chunk c] = K_j @ Q^T; on the
                        # diagonal band only cols >= 128r are causally live.
                        # bf16 has no small-N rate penalty, so trim all of it.
                        slo = P * r if r > 0 else 0
                        nc.tensor.matmul(
                            ps[:, slo:512],
                            kt[t][hp : hp + DH, P * j : P * (j + 1)],
                            qt[t][hp : hp + DH, q_lo + slo : q_hi],
                            start=True,
                            stop=True,
                        )
                        if r >= 0:
                            # triangle mask on the diagonal 128x128
                            nc.vector.tensor_add(
                                ps[:, P * r : P * (r + 1)],
                                ps[:, P * r : P * (r + 1)],
                                mask_sb[:],
                            )
                        pr = ppool.tile(
                            [P, 512], f32r, tag="probs", name=f"pr_{c}_{h}_{j}"
                        )
                        probs[(h, j)] = pr
                        lo = P * r if r > 0 else 0
                        nc.scalar.activation(
                            pr[:, lo:512], ps[:, lo:512], EXP, scale=SCALE
                        )
                    if j >= 1:
                        for h in heads:
                            av_mm(h, j - 1, first=(j == 1), last=False)
                for h in heads:
                    av_mm(h, nblk - 1, first=(nblk == 1), last=True)

                # stash unnormalized ctx + row sums
                for h in heads:
                    hp = DH * (h % 2)
                    nc.vector.tensor_copy(
                        ct[c][hp : hp + DH, t, :], ctx_ps[h][0:DH, :]
                    )
                    nc.vector.tensor_copy(
                        sums_sb[
                            64 * (t % 2) + 32 * (h % 2) : 64 * (t % 2) + 32 * (h % 2) + 1,
                            512 * (t // 2) : 512 * (t // 2) + 512,
                        ],
                        ctx_ps[h][DH : DH + 1, :],
                    )

            # ---- batched softmax normalization for this chunk: 1/sums as
            # exp(-ln(sums)) on ACT (the DVE-reciprocal variant measured
            # slower overall), written as bf16 directly where the broadcast
            # matmul reads it. bc matmuls are ordered by sums-row parity so
            # the all-ones stationary reloads dedup to one per row.
            lns = fpool.tile([P, 1024], f32, tag="lns", name=f"lns{c}", bufs=1)
            nc.scalar.activation(lns[:], sums_sb[:], LN)
            recs = fpool.tile([P, 1024], f32r, tag="recs", name=f"recs{c}")
            nc.scalar.activation(recs[:], lns[:], EXP, scale=-1.0)
            # one K=2 matmul per head PAIR: the block-indicator stationary
            # (bsel rows p%2==0 -> ones|zeros, p%2==1 -> zeros|ones) routes
            # rec row h_even to out partitions 0-63 and h_odd to 64-127, so
            # a single [128,512] psum serves both heads and one DVE mul
            # normalizes the whole pair slice. t-order (0,2,1,3) makes the
            # two bsel loads per base dedup.
            for t in (0, 2, 1, 3):
                rp = 64 * (t % 2)
                bc = ps_bc.tile([P, 512], f32, tag="ps_bc", name=f"bc{c}_{t}")
                nc.tensor.matmul(
                    bc[:],
                    bsel_sb[rp : rp + 33, :],
                    recs[rp : rp + 33, 512 * (t // 2) : 512 * (t // 2) + 512],
                    start=True,
                    stop=True,
                )
                nc.vector.tensor_mul(
                    ct[c][:, t, :],
                    ct[c][:, t, :],
                    bc[:],
                )


# ------------------------------------------------------------ PJRT runner
class _Runner:
    """Compile once, run many: mirrors bass2jax.run_bass_via_pjrt with a
    cached jitted executable."""

    def __init__(self, nc):
        import jax
        import jax.numpy  # noqa: F401
        from jax.sharding import Mesh, PartitionSpec
        from jax.experimental.shard_map import shard_map
        import concourse.bass2jax as b2j
        from concourse import mybir

        b2j.install_neuronx_cc_hook()
        self.jax = jax
        partition_name = (
            nc.partition_id_tensor.name if nc.partition_id_tensor else None
        )
        in_names = []
        out_names = []
        out_avals = []
        self.zero_shapes = []
        for alloc in nc.m.functions[0].allocations:
            if not isinstance(alloc, mybir.MemoryLocationSet):
                continue
            name = alloc.memorylocations[0].name
            if alloc.kind == "ExternalInput":
                if name == partition_name:
                    continue
                in_names.append(name)
            elif alloc.kind == "ExternalOutput":
                shape = tuple(alloc.tensor_shape)
                dtype = mybir.dt.np(alloc.dtype)
                out_names.append(name)
                out_avals.append(jax.core.ShapedArray(shape, dtype))
                self.zero_shapes.append((shape, dtype))
        self.in_names = in_names
        self.out_names = out_names
        self.out_avals = out_avals
        n_params = len(in_names)
        n_outs = len(out_avals)
        all_in = list(in_names) + list(out_names)
        if partition_name is not None:
            all_in.append(partition_name)

        def _body(*args):
            operands = list(args)
            if partition_name is not None:
                operands.append(b2j.partition_id_tensor())
            outs = b2j._bass_exec_p.bind(
                *operands,
                out_avals=tuple(out_avals),
                in_names=tuple(all_in),
                out_names=tuple(out_names),
                lowering_input_output_aliases=(),
                sim_require_finite=True,
                sim_require_nnan=True,
                nc=nc,
            )
            return tuple(outs)

        devices = jax.devices()[:NCORES]
        assert len(devices) == NCORES, f"need {NCORES} cores, got {len(devices)}"
        self.mesh = Mesh(np.asarray(devices), ("core",))
        in_specs = (PartitionSpec("core"),) * (n_params + n_outs)
        out_specs = (PartitionSpec("core"),) * n_outs
        self.fn = jax.jit(
            shard_map(
                _body,
                mesh=self.mesh,
                in_specs=in_specs,
                out_specs=out_specs,
                check_rep=False,
            ),
            donate_argnums=tuple(range(n_params, n_params + n_outs)),
            keep_unused=True,
        )

    def run(self, in_maps):
        concat_in = [
            np.concatenate([np.asarray(m[name]) for m in in_maps], axis=0)
            for name in self.in_names
        ]
        zeros = [
            np.zeros((NCORES * s[0], *s[1:]), dt) for s, dt in self.zero_shapes
        ]
        outs = self.fn(*concat_in, *zeros)
        return [
            {
                name: np.asarray(outs[i]).reshape(
                    NCORES, *self.out_avals[i].shape
                )[c]
                for i, name in enumerate(self.out_names)
            }
            for c in range(NCORES)
        ]


_cache = {}


def _get_runner():
    if "runner" not in _cache:
        _install_syncfix()
        _cache["runner"] = _Runner(build_nc())
    return _cache["runner"]


def make_in_maps(X, Wq, Wk, Wv, Wo, bo):
    import ml_dtypes

    bf16 = ml_dtypes.bfloat16
    X = np.asarray(X, dtype=np.float32)
    Wq = np.asarray(Wq, dtype=np.float32)
    Wk = np.asarray(Wk, dtype=np.float32)
    Wv = np.asarray(Wv, dtype=np.float32)
    Wo = np.asarray(Wo, dtype=np.float32)
    bo = np.asarray(bo, dtype=np.float32)

    kv = np.arange(P)[:, None]
    qq = np.arange(P)[None, :]
    mask = np.where(kv > qq, np.float32(NEG), np.float32(0.0))

    in_maps = []
    for core in range(NCORES):
        b, g = divmod(core, 2)
        h0 = HL * g
        in_maps.append(
            {
                "xt": np.ascontiguousarray(X[b].T.astype(bf16)),
                "wq": np.ascontiguousarray(
                    Wq[h0 : h0 + HL].transpose(1, 0, 2).reshape(E, DL).astype(bf16)
                ),
                "wk": np.ascontiguousarray(
                    Wk[h0 : h0 + HL].transpose(1, 0, 2).reshape(E, DL).astype(bf16)
                ),
                "wv": np.ascontiguousarray(
                    Wv[h0 : h0 + HL].transpose(1, 0, 2).reshape(E, DL).astype(bf16)
                ),
                "wo": np.ascontiguousarray(Wo[:, DL * g : DL * (g + 1)].T.astype(bf16)),
                "bo2": bo if g == 0 else np.zeros_like(bo),
                "mask": mask,
                "vones": np.ones((P, SB * HL), dtype=bf16),
                "o64": np.ones((P, DH), dtype=bf16),
                "bsel": np.stack(
                    [
                        np.concatenate([np.ones(DH), np.zeros(DH)])
                        if p in (0, 64)
                        else np.concatenate([np.zeros(DH), np.ones(DH)])
                        if p in (32, 96)
                        else np.zeros(P)
                        for p in range(P)
                    ]
                ).astype(bf16),
            }
        )
    return in_maps


def assemble(results):
    out = np.empty((B, S, E), dtype=np.float32)
    for b in range(B):
        acc = np.asarray(results[2 * b]["outp"], dtype=np.float32) + np.asarray(
            results[2 * b + 1]["outp"], dtype=np.float32
        )
        out[b] = acc.T
    return out


def kernel(X, Wq, Wk, Wv, Wo, bo):
    runner = _get_runner()
    in_maps = make_in_maps(X, Wq, Wk, Wv, Wo, bo)
    results = runner.run(in_maps)
    return assemble(results)
